# revision 8
# baseline (speedup 1.0000x reference)
"""Trainium2 Bass kernel for relational GNN message passing (BlockDecomposition).

Math (per reference): directed edges (both directions of each input edge)
carry messages m_e = x[src_e] @ blockdiag(blocks[rel_e]); out[t] = sum_e
w_e * m_e over edges with tgt_e == t.

v2 strategy (8 NeuronCores, SPMD, shared instruction stream). The HW
bottleneck is GPSIMD SWDGE descriptor generation (~8 ns per 256B row), so
the design minimizes indexed-DMA bytes to 2 x 256B per edge:
  - Partition output nodes across cores (12500 each); targets split into
    8 windows; edges sorted by (window, relation).
  - Host builds per-(core,window) ghost node tables (unique sources,
    bf16) uploaded as one DRAM input -> window-local int16 tokens, no
    on-device compaction pass.
  - Pass A per window: ONE transposed dma_gather per 2048 edges straight
    from the DRAM ghost table -> xT slabs (feat x edge). PE matmul with
    stationary xT chunk against dense 128x128 block-diagonal W_r ->
    msgs (edge x feat) in PSUM; Scalar-engine copy to bf16 staging;
    batched HWDGE writes to a DRAM msgs stream (128-padded chunks).
  - Pass B per window: dma_gather msgs rows by target-sorted rank
    (window-local int16), batched S-matrix build on DVE via two
    contention-free tensor_tensor ops (S[e,t] = (iota==tloc_e) * w_e),
    matmul-accumulate S^T @ msgs into PSUM per 128-target block, Scalar
    copy + HWDGE store.
  - Windows pipelined one ahead so GPSIMD gathers never wait on compute.
"""

import os
import sys

sys.path.insert(0, "/opt/trn_rl_repo")

import numpy as np
import ml_dtypes

_PATCHED = False


def _patch_tile_drain():
    """This container's walrus accepts at most one sync-wait per instruction,
    but TileContext's kernel-tail attaches every outstanding DMA-lane wait to
    a single Drain ("Too many sync wait commands"). Spread the waits across
    individual SP NOPs before the drain."""
    global _PATCHED
    if _PATCHED:
        return
    _PATCHED = True
    import concourse.mybir as mybir
    import concourse.tile as tile_mod
    from bass_rust import ScopedClock

    def _drain_and_barrier(self, tick_clock, wait_clock):
        nc = self.nc
        collector = nc.sync.nop(nofuse=True, hint="drain_waits")
        wait_clock.add_sem_waits(
            collector.ins, ScopedClock({None: tick_clock.global_clock})
        )
        si = collector.ins.sync_info
        waits = list(si.on_wait) if si and si.on_wait else []
        if len(waits) > 1:
            si.on_wait = waits[:1]
            for wv in waits[1:]:
                n2 = nc.sync.nop(nofuse=True, hint="drain_waits")
                n2.ins.sync_info = mybir.SyncInfo(on_wait=[wv], on_update=[])
        nc.sync.drain()
        nc.all_engine_barrier()
        assert self.sems is not None
        popped = nc._tile_sem_poison_stack.pop()
        assert popped is self._sem_poison
        nc.clear_and_free_semaphores(list(self.sems.allocated().values()))
        nc.all_engine_barrier()

    tile_mod.TileContext._drain_and_barrier = _drain_and_barrier

# ---------------- problem constants (hardcoded) ----------------
N_NODES = 100000
D = 128
R = 64           # relations used by edges (blocks table has R+1 rows)
NB = 8
BS = 16
N_EDGES = 500000
NCORES = 8
P = 128
NT = N_NODES // NCORES          # 12500 targets per core
TBLOCKS = (NT + P - 1) // P     # 98
NT_PAD = TBLOCKS * P            # 12544
N_WIN = 8
GOP = 2048                      # gather op size (edges / msgs rows)
SBATCH = 8                      # S-build batch (chunks)
MB_CH = 8                       # msgs HWDGE write batch (chunks)

TRACE = os.environ.get("GNN_TRACE", "0") == "1"
SIM = os.environ.get("GNN_SIM", "0") == "1"

bf16 = ml_dtypes.bfloat16

# window split of the 98 target blocks: two windows of 13, six of 12
_WSIZES = [13, 13, 12, 12, 12, 12, 12, 12]
assert sum(_WSIZES) == TBLOCKS
WIN_TB_START = np.cumsum([0] + _WSIZES)      # per-window first tblock
WIN_OF_TB = np.repeat(np.arange(N_WIN), _WSIZES)


def _idx_image(lst):
    """int16 index list (len % 128 == 0) -> SBUF image [128, len//16],
    entry i at (i%16, i//16), replicated across the 8 16-partition bands."""
    lst = np.asarray(lst, dtype=np.int16)
    n = len(lst)
    assert n % 128 == 0 and n > 0
    a = lst.reshape(n // 16, 16).T          # [16, n//16]
    return np.tile(a, (8, 1))               # [128, n//16]


def _ceil(a, b):
    return -(-a // b)


def _round_up(a, b):
    return _ceil(a, b) * b


def _preprocess(x, blocks, edge_weights, source, target, edge_type):
    """Host-side: build all per-core device inputs + the shared structure."""
    src = np.asarray(source).astype(np.int64)
    tgt = np.asarray(target).astype(np.int64)
    rel = np.asarray(edge_type).astype(np.int64)
    w = np.asarray(edge_weights).astype(np.float32)

    # directed edges (both directions, same relation/weight)
    s2 = np.concatenate([src, tgt])
    t2 = np.concatenate([tgt, src])
    r2 = np.concatenate([rel, rel])
    w2 = np.concatenate([w, w])

    owner = t2 // NT
    tloc = t2 - owner * NT
    tb = tloc // P
    win = WIN_OF_TB[tb]

    # ---- per-core edge lists sorted by (window, relation) ----
    cores = []
    for c in range(NCORES):
        m = owner == c
        key = win[m] * R + r2[m]
        order = np.argsort(key, kind="stable")
        cores.append({
            "s": s2[m][order],
            "tloc": tloc[m][order],
            "r": r2[m][order],
            "w": w2[m][order],
            "key": key[order],
        })

    # group sizes n[c, w*R+r]; shared caps
    NG = N_WIN * R
    n_grp = np.zeros((NCORES, NG), np.int64)
    for c in range(NCORES):
        n_grp[c] = np.bincount(cores[c]["key"], minlength=NG)
    cap = n_grp.max(axis=0)                       # shared group capacity

    # ---- ghost tables: unique sources per (core, window); shared sizes ----
    uniq_per_cw = [[None] * N_WIN for _ in range(NCORES)]
    uniq_cnt = np.zeros((NCORES, N_WIN), np.int64)
    for c in range(NCORES):
        cw = cores[c]
        gwin = cw["key"] // R
        for wv in range(N_WIN):
            uu = np.unique(cw["s"][gwin == wv])    # sorted ascending
            uniq_per_cw[c][wv] = uu
            uniq_cnt[c, wv] = len(uu)
    CT = _round_up(np.maximum(uniq_cnt.max(axis=0), 1), P)   # shared per win
    XB = np.concatenate([[0], np.cumsum(CT)])     # ghost-table window bases
    XG_TOT = int(XB[-1])
    assert CT.max() < 32768

    # ---- shared pass-A chunk structure ----
    # msgs layout: chunk-major stream, chunk i occupies rows [128i, 128i+128)
    # Per (win, rel) group: ceil(cap/128) chunks, possibly split at slab
    # (GOP-column) boundaries of the edge-gather list.
    EWpad = np.zeros(N_WIN, np.int64)             # gather list len per window
    for wv in range(N_WIN):
        EWpad[wv] = _round_up(max(int(cap[wv * R:(wv + 1) * R].sum()), 1), GOP)

    p1 = [[] for _ in range(N_WIN)]   # (slab, scol, n, rel) per chunk
    grp_chunk0 = np.zeros(NG + 1, np.int64)       # first chunk id of group
    # edge-gather-list position of each group (within window, padded by cap)
    ch_cursor = 0
    for wv in range(N_WIN):
        ecursor = 0
        for r in range(R):
            g = wv * R + r
            grp_chunk0[g] = ch_cursor
            cgap = int(cap[g])
            done = 0
            while done < cgap:
                sl = (ecursor + done) // GOP
                scol = (ecursor + done) % GOP
                n = min(cgap - done, P, GOP - scol)
                p1[wv].append((sl, scol, int(n), r))
                ch_cursor += 1
                done += n
            ecursor += cgap
    grp_chunk0[NG] = ch_cursor
    TOT_CHUNKS = int(ch_cursor)
    MSG_ROWS = TOT_CHUNKS * P
    # per-window msgs base/extent in the chunk stream
    WIN_CH0 = [int(grp_chunk0[wv * R]) for wv in range(N_WIN)]
    WIN_CH0.append(TOT_CHUNKS)
    for wv in range(N_WIN):
        assert (WIN_CH0[wv + 1] - WIN_CH0[wv]) * P < 32768, "msgs win > int16"

    # map: chunk id -> (win, slab, scol, n) and msg row base = 128*chunk
    # build per-edge msg position for pass 2:
    # edges of group g sit at chunks grp_chunk0[g].. with chunk fills n_i.

    # ---- pass-2 shared structure (target-block caps) ----
    n_tb = np.zeros((NCORES, TBLOCKS), np.int64)
    for c in range(NCORES):
        n_tb[c] = np.bincount(cores[c]["tloc"] // P, minlength=TBLOCKS)
    cap_tb = np.maximum(n_tb.max(axis=0), 1)
    chunks_tb = _ceil(cap_tb, P)                   # pass-2 chunks per tblock
    slots_tb = chunks_tb * P
    win_chunks = [int(chunks_tb[WIN_TB_START[wv]:WIN_TB_START[wv + 1]].sum())
                  for wv in range(N_WIN)]
    win_slots = [_round_up(ch * P, GOP) for ch in win_chunks]  # gather len

    # ---- per-core arrays ----
    eidx_cols = []   # edge xT gather images (tokens)
    midx_cols = []   # pass-2 msg gather images (window-local msg rows)
    tgtw_arrs = []   # pass-2 [chunks, 128, 2] bf16 meta (tloc, w)
    xg_rows = []     # ghost table rows (bf16)

    xbf = np.asarray(x, dtype=np.float32).astype(bf16)

    for c in range(NCORES):
        cw = cores[c]
        gwin = cw["key"] // R
        e_img, m_img = [], []
        tg_meta = []
        xg = np.zeros((XG_TOT, D), bf16)
        for wv in range(N_WIN):
            uu = uniq_per_cw[c][wv]
            xg[XB[wv]:XB[wv] + len(uu)] = xbf[uu]
            # token id of each source node in this window's ghost table
            lut = np.zeros(N_NODES, np.int64)
            lut[uu] = np.arange(len(uu))
            wmask = gwin == wv
            ws = cw["s"][wmask]
            wr = cw["r"][wmask]
            wt = cw["tloc"][wmask]
            ww = cw["w"][wmask]
            elist = np.zeros(int(EWpad[wv]), np.int64)
            # per-edge msg position (window-local)
            pos_in_win = np.zeros(len(ws), np.int64)
            ecursor = 0
            for r in range(R):
                g = wv * R + r
                gsel = wr == r
                cnt = int(gsel.sum())
                if cnt:
                    elist[ecursor:ecursor + cnt] = lut[ws[gsel]]
                    # msg position: walk this group's chunks
                    pos = np.zeros(cnt, np.int64)
                    ci = grp_chunk0[g]
                    left = 0
                    gdone = 0
                    cgap = int(cap[g])
                    while gdone < cgap and left < cnt:
                        sl = (ecursor + gdone) % GOP
                        n = min(cgap - gdone, P, GOP - sl)
                        take = min(n, cnt - left)
                        base = (ci - WIN_CH0[wv]) * P
                        pos[left:left + take] = base + np.arange(take)
                        left += take
                        gdone += n
                        ci += 1
                    pos_in_win[gsel] = pos
                ecursor += int(cap[g])
            e_img.append(elist)
            # pass-2: sort window's real edges by target
            o2 = np.argsort(wt, kind="stable")
            wt2, ww2, pos2 = wt[o2], ww[o2], pos_in_win[o2]
            tb2 = wt2 // P
            mlist = np.zeros(win_slots[wv], np.int64)
            meta = np.zeros((win_chunks[wv], P, 2), np.float32)
            scursor = 0
            for tbi in range(WIN_TB_START[wv], WIN_TB_START[wv + 1]):
                sel = tb2 == tbi
                cnt = int(sel.sum())
                nslots = int(slots_tb[tbi])
                mlist[scursor:scursor + cnt] = pos2[sel]
                mview = meta.reshape(-1, 2)
                mview[scursor:scursor + cnt, 0] = (
                    wt2[sel] - tbi * P).astype(np.float32)
                mview[scursor:scursor + cnt, 1] = ww2[sel]
                # pad slots: idx 0, tgt 0, w 0 (already zeros)
                scursor += nslots
            m_img.append(mlist)
            tg_meta.append(meta)
        eidx_cols.append(_idx_image(np.concatenate(e_img)))
        midx_cols.append(_idx_image(np.concatenate(m_img)))
        tgtw_arrs.append(np.concatenate(tg_meta, axis=0).astype(bf16))
        xg_rows.append(xg)

    shared = {
        "CT": CT, "XB": XB, "XG_TOT": XG_TOT,
        "EWpad": EWpad, "p1": p1,
        "WIN_CH0": WIN_CH0, "TOT_CHUNKS": TOT_CHUNKS, "MSG_ROWS": MSG_ROWS,
        "chunks_tb": chunks_tb, "win_chunks": win_chunks,
        "win_slots": win_slots,
        "eidx_w": eidx_cols[0].shape[1],
        "midx_w": midx_cols[0].shape[1],
        "tgtw_n": tgtw_arrs[0].shape[0],
    }

    # dense block-diagonal weights, (i, r*128+j), bf16
    wd = np.zeros((P, (R + 1) * P), dtype=bf16)
    blk = np.asarray(blocks, dtype=np.float32)
    for r in range(R):
        for b in range(NB):
            wd[b * BS:(b + 1) * BS, r * P + b * BS:r * P + (b + 1) * BS] = \
                blk[r, b].astype(bf16)
    iota = np.broadcast_to(np.arange(P, dtype=np.float32),
                           (P, P)).astype(bf16).copy()

    in_maps = []
    for c in range(NCORES):
        in_maps.append({
            "xg": xg_rows[c],
            "wd": wd,
            "iota": iota,
            "eidx": eidx_cols[c],
            "midx": midx_cols[c],
            "tgtw": tgtw_arrs[c],
        })
    return shared, in_maps


def _build_nc(shared):
    _patch_tile_drain()
    import concourse.bacc as bacc
    import concourse.mybir as mybir
    from concourse.tile import TileContext

    CT, XB = shared["CT"], shared["XB"]
    EWpad = shared["EWpad"]
    p1 = shared["p1"]
    WIN_CH0 = shared["WIN_CH0"]
    chunks_tb = shared["chunks_tb"]
    win_chunks = shared["win_chunks"]
    win_slots = shared["win_slots"]

    f32 = mybir.dt.float32
    bf = mybir.dt.bfloat16
    i16 = mybir.dt.int16

    nc = bacc.Bacc("TRN2", target_bir_lowering=False, debug=False)
    xg_d = nc.dram_tensor("xg", [shared["XG_TOT"], D], bf,
                          kind="ExternalInput")
    wd_d = nc.dram_tensor("wd", [P, (R + 1) * P], bf, kind="ExternalInput")
    iota_d = nc.dram_tensor("iota", [P, P], bf, kind="ExternalInput")
    eidx_d = nc.dram_tensor("eidx", [P, shared["eidx_w"]], i16,
                            kind="ExternalInput")
    midx_d = nc.dram_tensor("midx", [P, shared["midx_w"]], i16,
                            kind="ExternalInput")
    tgtw_d = nc.dram_tensor("tgtw", [shared["tgtw_n"], P, 2], bf,
                            kind="ExternalInput")
    out_d = nc.dram_tensor("out", [NT_PAD, D], f32, kind="ExternalOutput")
    msgs_d = nc.dram_tensor("msgs", [max(shared["MSG_ROWS"], P), D], bf,
                            kind="Internal")

    max_meta = max(max(win_chunks), 1) * 2

    with TileContext(nc) as tc:
        with (
            tc.tile_pool(name="cpool", bufs=1) as cpool,
            tc.tile_pool(name="xtp", bufs=3) as xtp,
            tc.tile_pool(name="msp", bufs=3) as msp,
            tc.tile_pool(name="stg", bufs=2) as stg,
            tc.tile_pool(name="metap", bufs=2) as metap,
            tc.tile_pool(name="spool", bufs=2) as spool,
            tc.tile_pool(name="outp", bufs=4) as outp,
            tc.tile_pool(name="psA", bufs=4, space="PSUM") as psA,
            tc.tile_pool(name="psB", bufs=2, space="PSUM") as psB,
        ):
            wd_t = cpool.tile([P, (R + 1) * P], bf)
            iota_t = cpool.tile([P, P], bf)
            eidx_t = cpool.tile([P, shared["eidx_w"]], i16)
            midx_t = cpool.tile([P, shared["midx_w"]], i16)
            nc.sync.dma_start(out=wd_t[:], in_=wd_d[:])
            nc.sync.dma_start(out=iota_t[:], in_=iota_d[:])
            nc.sync.dma_start(out=eidx_t[:], in_=eidx_d[:])
            nc.sync.dma_start(out=midx_t[:], in_=midx_d[:])
            iota_b = iota_t[:].rearrange("p (c f) -> p c f", c=1)

            # per-window emission, pass-A one window ahead of pass-B
            slabs_w = {}     # wv -> list of slab tiles

            def emit_passA(wv):
                nops = int(EWpad[wv]) // GOP
                ecol0 = int(EWpad[:wv].sum()) // 16
                slabs = []
                for k in range(nops):
                    xts = xtp.tile([P, GOP], bf, tag="xts",
                                   name=f"xts_{wv}_{k}")
                    nc.gpsimd.dma_gather(
                        out_ap=xts[:].rearrange("p (c e) -> p c e", e=GOP),
                        in_ap=xg_d[int(XB[wv]):, :],
                        idxs_ap=eidx_t[:, ecol0 + k * GOP // 16:
                                       ecol0 + (k + 1) * GOP // 16],
                        num_idxs=GOP, num_idxs_reg=GOP, elem_size=D,
                        single_packet=False, transpose=True,
                    )
                    slabs.append(xts)
                slabs_w[wv] = slabs
                # matmuls + staging copies + batched msgs writes
                chunks = p1[wv]
                ch0 = WIN_CH0[wv]
                i = 0
                while i < len(chunks):
                    nb = min(MB_CH, len(chunks) - i)
                    st = stg.tile([P, MB_CH * P], bf, tag="stage",
                                  name=f"stg_{wv}_{i}")
                    if SIM:
                        # sim shadow-memory: partial chunks (n<128) leave
                        # staging rows unwritten; harmless garbage on HW.
                        nc.vector.memset(st[:], 0.0)
                    for j in range(nb):
                        sl, scol, n, r = chunks[i + j]
                        mp = psA.tile([P, P], f32, tag="mp",
                                      name=f"mp_{wv}_{i}_{j}")
                        nc.tensor.matmul(
                            out=mp[:n, :],
                            lhsT=slabs[sl][:, scol:scol + n],
                            rhs=wd_t[:, r * P:(r + 1) * P],
                            start=True, stop=True,
                        )
                        nc.scalar.copy(out=st[:n, j * P:(j + 1) * P],
                                       in_=mp[:n, :])
                    base = (ch0 + i) * P
                    nc.sync.dma_start(
                        out=msgs_d[base:base + nb * P, :].rearrange(
                            "(c p) f -> p c f", p=P),
                        in_=st[:, :nb * P].rearrange("p (c f) -> p c f", f=P),
                    )
                    i += nb

            def emit_passB(wv):
                wch = win_chunks[wv]
                mcol0 = int(sum(win_slots[:wv])) // 16
                chb = int(sum(win_chunks[:wv]))
                meta_t = metap.tile([P, max_meta], bf, tag="meta",
                                    name=f"meta_{wv}")
                nc.sync.dma_start(
                    out=meta_t[:, :wch * 2].rearrange("p (c k) -> p c k", k=2),
                    in_=tgtw_d[chb:chb + wch].rearrange("c p k -> p c k"),
                )
                meta_b = meta_t[:].rearrange("p (c k) -> p c k", k=2)
                mslabs = []
                for k in range(win_slots[wv] // GOP):
                    msl = msp.tile([P, GOP], bf, tag="msl",
                                   name=f"msl_{wv}_{k}")
                    nc.gpsimd.dma_gather(
                        out_ap=msl[:].rearrange("p (c e) -> p c e", e=D),
                        in_ap=msgs_d[WIN_CH0[wv] * P:WIN_CH0[wv + 1] * P, :],
                        idxs_ap=midx_t[:, mcol0 + k * GOP // 16:
                                       mcol0 + (k + 1) * GOP // 16],
                        num_idxs=GOP, num_idxs_reg=GOP, elem_size=D,
                        single_packet=False,
                    )
                    mslabs.append(msl)

                # batched S builds + per-tb accumulation
                ci = 0
                S_t = None
                for tbi in range(WIN_TB_START[wv], WIN_TB_START[wv + 1]):
                    K = int(chunks_tb[tbi])
                    acc = psB.tile([P, P], f32, tag="acc",
                                   name=f"acc_{wv}_{tbi}")
                    for k in range(K):
                        if ci % SBATCH == 0:
                            nbs = min(SBATCH, wch - ci)
                            S_t = spool.tile([P, SBATCH * P], bf, tag="S",
                                             name=f"S_{wv}_{ci}")
                            Sb = S_t[:, :nbs * P].rearrange(
                                "p (c f) -> p c f", f=P)
                            nc.vector.tensor_tensor(
                                out=Sb,
                                in0=iota_b.to_broadcast([P, nbs, P]),
                                in1=meta_b[:, ci:ci + nbs, 0:1].to_broadcast(
                                    [P, nbs, P]),
                                op=mybir.AluOpType.is_equal,
                            )
                            nc.vector.tensor_tensor(
                                out=Sb,
                                in0=Sb,
                                in1=meta_b[:, ci:ci + nbs, 1:2].to_broadcast(
                                    [P, nbs, P]),
                                op=mybir.AluOpType.mult,
                            )
                        sslot = ci % SBATCH
                        gpos = ci * P
                        sl, scol = gpos // GOP, gpos % GOP
                        nc.tensor.matmul(
                            out=acc[:],
                            lhsT=S_t[:, sslot * P:(sslot + 1) * P],
                            rhs=mslabs[sl][:, scol:scol + P],
                            start=(k == 0), stop=(k == K - 1),
                        )
                        ci += 1
                    ot = outp.tile([P, P], f32, tag="ot",
                                   name=f"ot_{wv}_{tbi}")
                    nc.scalar.copy(out=ot[:], in_=acc[:])
                    nc.sync.dma_start(out=out_d[tbi * P:(tbi + 1) * P, :],
                                      in_=ot[:])

            for wv in range(N_WIN):
                emit_passA(wv)
                if wv >= 1:
                    emit_passB(wv - 1)
            emit_passB(N_WIN - 1)
    nc.finalize()
    return nc


def kernel(x, blocks, edge_weights, source, target, edge_type):
    from concourse import bass_utils

    shared, in_maps = _preprocess(x, blocks, edge_weights, source, target,
                                  edge_type)
    nc = _build_nc(shared)
    if SIM:
        from concourse.bass_interp import CoreSim
        sim = CoreSim(nc, trace=False)
        for k, v in in_maps[0].items():
            sim.tensor(k)[:] = v
        sim.simulate()
        out = np.asarray(sim.tensor("out"))[:NT]
        # single core only in sim: return zeros elsewhere
        full = np.zeros((N_NODES, D), np.float32)
        full[:NT] = out
        kernel.last_exec_ns = None
        return full
    res = bass_utils.run_bass_kernel_spmd(
        nc, in_maps, core_ids=list(range(NCORES)), trace=TRACE,
    )
    out = np.concatenate([res.results[c]["out"][:NT] for c in range(NCORES)],
                         axis=0)
    if TRACE:
        kernel.last_exec_ns = res.exec_time_ns
    return out.astype(np.float32)


kernel.last_exec_ns = None


# revision 9
# speedup vs baseline: 1.1666x; 1.1666x over previous
"""Trainium2 Bass kernel for relational GNN message passing (BlockDecomposition).

Math (per reference): directed edges (both directions of each input edge)
carry messages m_e = x[src_e] @ blockdiag(blocks[rel_e]); out[t] = sum_e
w_e * m_e over edges with tgt_e == t.

v2 strategy (8 NeuronCores, SPMD, shared instruction stream). The HW
bottleneck is GPSIMD SWDGE descriptor generation (~8 ns per 256B row), so
the design minimizes indexed-DMA bytes to 2 x 256B per edge:
  - Partition output nodes across cores (12500 each); targets split into
    8 windows; edges sorted by (window, relation).
  - Host builds per-(core,window) ghost node tables (unique sources,
    bf16) uploaded as one DRAM input -> window-local int16 tokens, no
    on-device compaction pass.
  - Pass A per window: ONE transposed dma_gather per 2048 edges straight
    from the DRAM ghost table -> xT slabs (feat x edge). PE matmul with
    stationary xT chunk against dense 128x128 block-diagonal W_r ->
    msgs (edge x feat) in PSUM; Scalar-engine copy to bf16 staging;
    batched HWDGE writes to a DRAM msgs stream (128-padded chunks).
  - Pass B per window: dma_gather msgs rows by target-sorted rank
    (window-local int16), batched S-matrix build on DVE via two
    contention-free tensor_tensor ops (S[e,t] = (iota==tloc_e) * w_e),
    matmul-accumulate S^T @ msgs into PSUM per 128-target block, Scalar
    copy + HWDGE store.
  - Windows pipelined one ahead so GPSIMD gathers never wait on compute.
"""

import os
import sys

sys.path.insert(0, "/opt/trn_rl_repo")

import numpy as np
import ml_dtypes

_PATCHED = False


def _patch_tile_drain():
    """This container's walrus accepts at most one sync-wait per instruction,
    but TileContext's kernel-tail attaches every outstanding DMA-lane wait to
    a single Drain ("Too many sync wait commands"). Spread the waits across
    individual SP NOPs before the drain."""
    global _PATCHED
    if _PATCHED:
        return
    _PATCHED = True
    import concourse.mybir as mybir
    import concourse.tile as tile_mod
    from bass_rust import ScopedClock

    def _drain_and_barrier(self, tick_clock, wait_clock):
        nc = self.nc
        collector = nc.sync.nop(nofuse=True, hint="drain_waits")
        wait_clock.add_sem_waits(
            collector.ins, ScopedClock({None: tick_clock.global_clock})
        )
        si = collector.ins.sync_info
        waits = list(si.on_wait) if si and si.on_wait else []
        if len(waits) > 1:
            si.on_wait = waits[:1]
            for wv in waits[1:]:
                n2 = nc.sync.nop(nofuse=True, hint="drain_waits")
                n2.ins.sync_info = mybir.SyncInfo(on_wait=[wv], on_update=[])
        nc.sync.drain()
        nc.all_engine_barrier()
        assert self.sems is not None
        popped = nc._tile_sem_poison_stack.pop()
        assert popped is self._sem_poison
        nc.clear_and_free_semaphores(list(self.sems.allocated().values()))
        nc.all_engine_barrier()

    tile_mod.TileContext._drain_and_barrier = _drain_and_barrier

# ---------------- problem constants (hardcoded) ----------------
N_NODES = 100000
D = 128
R = 64           # relations used by edges (blocks table has R+1 rows)
NB = 8
BS = 16
N_EDGES = 500000
NCORES = 8
P = 128
NT = N_NODES // NCORES          # 12500 targets per core
TBLOCKS = (NT + P - 1) // P     # 98
NT_PAD = TBLOCKS * P            # 12544
N_WIN = 8
GOP = 2048                      # gather op size (edges / msgs rows)
SBATCH = 8                      # S-build batch (chunks)
MB_CH = 8                       # msgs HWDGE write batch (chunks)

TRACE = os.environ.get("GNN_TRACE", "0") == "1"
SIM = os.environ.get("GNN_SIM", "0") == "1"

bf16 = ml_dtypes.bfloat16

# window split of the 98 target blocks: two windows of 13, six of 12
_WSIZES = [13, 13, 12, 12, 12, 12, 12, 12]
assert sum(_WSIZES) == TBLOCKS
WIN_TB_START = np.cumsum([0] + _WSIZES)      # per-window first tblock
WIN_OF_TB = np.repeat(np.arange(N_WIN), _WSIZES)


def _idx_image(lst):
    """int16 index list (len % 128 == 0) -> SBUF image [128, len//16],
    entry i at (i%16, i//16), replicated across the 8 16-partition bands."""
    lst = np.asarray(lst, dtype=np.int16)
    n = len(lst)
    assert n % 128 == 0 and n > 0
    a = lst.reshape(n // 16, 16).T          # [16, n//16]
    return np.tile(a, (8, 1))               # [128, n//16]


def _ceil(a, b):
    return -(-a // b)


def _round_up(a, b):
    return _ceil(a, b) * b


def _preprocess(x, blocks, edge_weights, source, target, edge_type):
    """Host-side: build all per-core device inputs + the shared structure."""
    src = np.asarray(source).astype(np.int64)
    tgt = np.asarray(target).astype(np.int64)
    rel = np.asarray(edge_type).astype(np.int64)
    w = np.asarray(edge_weights).astype(np.float32)

    # directed edges (both directions, same relation/weight)
    s2 = np.concatenate([src, tgt])
    t2 = np.concatenate([tgt, src])
    r2 = np.concatenate([rel, rel])
    w2 = np.concatenate([w, w])

    owner = t2 // NT
    tloc = t2 - owner * NT
    tb = tloc // P
    win = WIN_OF_TB[tb]

    # ---- per-core edge lists sorted by (window, relation) ----
    cores = []
    for c in range(NCORES):
        m = owner == c
        key = win[m] * R + r2[m]
        order = np.argsort(key, kind="stable")
        cores.append({
            "s": s2[m][order],
            "tloc": tloc[m][order],
            "r": r2[m][order],
            "w": w2[m][order],
            "key": key[order],
        })

    # group sizes n[c, w*R+r]; shared caps
    NG = N_WIN * R
    n_grp = np.zeros((NCORES, NG), np.int64)
    for c in range(NCORES):
        n_grp[c] = np.bincount(cores[c]["key"], minlength=NG)
    cap = n_grp.max(axis=0)                       # shared group capacity

    # ---- ghost tables: unique sources per (core, window); shared sizes ----
    uniq_per_cw = [[None] * N_WIN for _ in range(NCORES)]
    uniq_cnt = np.zeros((NCORES, N_WIN), np.int64)
    for c in range(NCORES):
        cw = cores[c]
        gwin = cw["key"] // R
        for wv in range(N_WIN):
            uu = np.unique(cw["s"][gwin == wv])    # sorted ascending
            uniq_per_cw[c][wv] = uu
            uniq_cnt[c, wv] = len(uu)
    CT = _round_up(np.maximum(uniq_cnt.max(axis=0), 1), P)   # shared per win
    XB = np.concatenate([[0], np.cumsum(CT)])     # ghost-table window bases
    XG_TOT = int(XB[-1])
    assert CT.max() < 32768

    # ---- shared pass-A chunk structure ----
    # msgs layout: chunk-major stream, chunk i occupies rows [128i, 128i+128)
    # Per (win, rel) group: ceil(cap/128) chunks, possibly split at slab
    # (GOP-column) boundaries of the edge-gather list.
    EWpad = np.zeros(N_WIN, np.int64)             # gather list len per window
    for wv in range(N_WIN):
        EWpad[wv] = _round_up(max(int(cap[wv * R:(wv + 1) * R].sum()), 1), P)

    p1 = [[] for _ in range(N_WIN)]   # (slab, scol, n, rel) per chunk
    grp_chunk0 = np.zeros(NG + 1, np.int64)       # first chunk id of group
    # edge-gather-list position of each group (within window, padded by cap)
    ch_cursor = 0
    for wv in range(N_WIN):
        ecursor = 0
        for r in range(R):
            g = wv * R + r
            grp_chunk0[g] = ch_cursor
            cgap = int(cap[g])
            done = 0
            while done < cgap:
                sl = (ecursor + done) // GOP
                scol = (ecursor + done) % GOP
                n = min(cgap - done, P, GOP - scol)
                p1[wv].append((sl, scol, int(n), r))
                ch_cursor += 1
                done += n
            ecursor += cgap
    grp_chunk0[NG] = ch_cursor
    TOT_CHUNKS = int(ch_cursor)
    MSG_ROWS = TOT_CHUNKS * P
    # per-window msgs base/extent in the chunk stream
    WIN_CH0 = [int(grp_chunk0[wv * R]) for wv in range(N_WIN)]
    WIN_CH0.append(TOT_CHUNKS)
    for wv in range(N_WIN):
        assert (WIN_CH0[wv + 1] - WIN_CH0[wv]) * P < 32768, "msgs win > int16"

    # map: chunk id -> (win, slab, scol, n) and msg row base = 128*chunk
    # build per-edge msg position for pass 2:
    # edges of group g sit at chunks grp_chunk0[g].. with chunk fills n_i.

    # ---- pass-2 shared structure (target-block caps) ----
    n_tb = np.zeros((NCORES, TBLOCKS), np.int64)
    for c in range(NCORES):
        n_tb[c] = np.bincount(cores[c]["tloc"] // P, minlength=TBLOCKS)
    cap_tb = np.maximum(n_tb.max(axis=0), 1)
    chunks_tb = _ceil(cap_tb, P)                   # pass-2 chunks per tblock
    slots_tb = chunks_tb * P
    win_chunks = [int(chunks_tb[WIN_TB_START[wv]:WIN_TB_START[wv + 1]].sum())
                  for wv in range(N_WIN)]
    win_slots = [ch * P for ch in win_chunks]  # gather list len

    # ---- per-core arrays ----
    eidx_cols = []   # edge xT gather images (tokens)
    midx_cols = []   # pass-2 msg gather images (window-local msg rows)
    tgtw_arrs = []   # pass-2 [chunks, 128, 2] bf16 meta (tloc, w)
    xg_rows = []     # ghost table rows (bf16)

    xbf = np.asarray(x, dtype=np.float32).astype(bf16)

    for c in range(NCORES):
        cw = cores[c]
        gwin = cw["key"] // R
        e_img, m_img = [], []
        tg_meta = []
        xg = np.zeros((XG_TOT, D), bf16)
        for wv in range(N_WIN):
            uu = uniq_per_cw[c][wv]
            xg[XB[wv]:XB[wv] + len(uu)] = xbf[uu]
            # token id of each source node in this window's ghost table
            lut = np.zeros(N_NODES, np.int64)
            lut[uu] = np.arange(len(uu))
            wmask = gwin == wv
            ws = cw["s"][wmask]
            wr = cw["r"][wmask]
            wt = cw["tloc"][wmask]
            ww = cw["w"][wmask]
            elist = np.zeros(int(EWpad[wv]), np.int64)
            # per-edge msg position (window-local)
            pos_in_win = np.zeros(len(ws), np.int64)
            ecursor = 0
            for r in range(R):
                g = wv * R + r
                gsel = wr == r
                cnt = int(gsel.sum())
                if cnt:
                    elist[ecursor:ecursor + cnt] = lut[ws[gsel]]
                    # msg position: walk this group's chunks
                    pos = np.zeros(cnt, np.int64)
                    ci = grp_chunk0[g]
                    left = 0
                    gdone = 0
                    cgap = int(cap[g])
                    while gdone < cgap and left < cnt:
                        sl = (ecursor + gdone) % GOP
                        n = min(cgap - gdone, P, GOP - sl)
                        take = min(n, cnt - left)
                        base = (ci - WIN_CH0[wv]) * P
                        pos[left:left + take] = base + np.arange(take)
                        left += take
                        gdone += n
                        ci += 1
                    pos_in_win[gsel] = pos
                ecursor += int(cap[g])
            e_img.append(elist)
            # pass-2: sort window's real edges by target
            o2 = np.argsort(wt, kind="stable")
            wt2, ww2, pos2 = wt[o2], ww[o2], pos_in_win[o2]
            tb2 = wt2 // P
            mlist = np.zeros(win_slots[wv], np.int64)
            meta = np.zeros((win_chunks[wv], P, 2), np.float32)
            scursor = 0
            for tbi in range(WIN_TB_START[wv], WIN_TB_START[wv + 1]):
                sel = tb2 == tbi
                cnt = int(sel.sum())
                nslots = int(slots_tb[tbi])
                mlist[scursor:scursor + cnt] = pos2[sel]
                mview = meta.reshape(-1, 2)
                mview[scursor:scursor + cnt, 0] = (
                    wt2[sel] - tbi * P).astype(np.float32)
                mview[scursor:scursor + cnt, 1] = ww2[sel]
                # pad slots: idx 0, tgt 0, w 0 (already zeros)
                scursor += nslots
            m_img.append(mlist)
            tg_meta.append(meta)
        eidx_cols.append(_idx_image(np.concatenate(e_img)))
        midx_cols.append(_idx_image(np.concatenate(m_img)))
        tgtw_arrs.append(np.concatenate(tg_meta, axis=0).astype(bf16))
        xg_rows.append(xg)

    shared = {
        "CT": CT, "XB": XB, "XG_TOT": XG_TOT,
        "EWpad": EWpad, "p1": p1,
        "WIN_CH0": WIN_CH0, "TOT_CHUNKS": TOT_CHUNKS, "MSG_ROWS": MSG_ROWS,
        "chunks_tb": chunks_tb, "win_chunks": win_chunks,
        "win_slots": win_slots,
        "eidx_w": eidx_cols[0].shape[1],
        "midx_w": midx_cols[0].shape[1],
        "tgtw_n": tgtw_arrs[0].shape[0],
    }

    # dense block-diagonal weights, (i, r*128+j), bf16
    wd = np.zeros((P, (R + 1) * P), dtype=bf16)
    blk = np.asarray(blocks, dtype=np.float32)
    for r in range(R):
        for b in range(NB):
            wd[b * BS:(b + 1) * BS, r * P + b * BS:r * P + (b + 1) * BS] = \
                blk[r, b].astype(bf16)
    iota = np.broadcast_to(np.arange(P, dtype=np.float32),
                           (P, P)).astype(bf16).copy()

    in_maps = []
    for c in range(NCORES):
        in_maps.append({
            "xg": xg_rows[c],
            "wd": wd,
            "iota": iota,
            "eidx": eidx_cols[c],
            "midx": midx_cols[c],
            "tgtw": tgtw_arrs[c],
        })
    return shared, in_maps


def _build_nc(shared):
    _patch_tile_drain()
    import concourse.bacc as bacc
    import concourse.mybir as mybir
    from concourse.tile import TileContext

    CT, XB = shared["CT"], shared["XB"]
    EWpad = shared["EWpad"]
    p1 = shared["p1"]
    WIN_CH0 = shared["WIN_CH0"]
    chunks_tb = shared["chunks_tb"]
    win_chunks = shared["win_chunks"]
    win_slots = shared["win_slots"]

    f32 = mybir.dt.float32
    bf = mybir.dt.bfloat16
    i16 = mybir.dt.int16

    nc = bacc.Bacc("TRN2", target_bir_lowering=False, debug=False)
    xg_d = nc.dram_tensor("xg", [shared["XG_TOT"], D], bf,
                          kind="ExternalInput")
    wd_d = nc.dram_tensor("wd", [P, (R + 1) * P], bf, kind="ExternalInput")
    iota_d = nc.dram_tensor("iota", [P, P], bf, kind="ExternalInput")
    eidx_d = nc.dram_tensor("eidx", [P, shared["eidx_w"]], i16,
                            kind="ExternalInput")
    midx_d = nc.dram_tensor("midx", [P, shared["midx_w"]], i16,
                            kind="ExternalInput")
    tgtw_d = nc.dram_tensor("tgtw", [shared["tgtw_n"], P, 2], bf,
                            kind="ExternalInput")
    out_d = nc.dram_tensor("out", [NT_PAD, D], f32, kind="ExternalOutput")
    msgs_d = nc.dram_tensor("msgs", [max(shared["MSG_ROWS"], P), D], bf,
                            kind="Internal")

    max_meta = max(max(win_chunks), 1) * 2

    with TileContext(nc) as tc:
        with (
            tc.tile_pool(name="cpool", bufs=1) as cpool,
            tc.tile_pool(name="xtp", bufs=4) as xtp,
            tc.tile_pool(name="msp", bufs=4) as msp,
            tc.tile_pool(name="stg", bufs=2) as stg,
            tc.tile_pool(name="metap", bufs=2) as metap,
            tc.tile_pool(name="spool", bufs=2) as spool,
            tc.tile_pool(name="outp", bufs=4) as outp,
            tc.tile_pool(name="psA", bufs=4, space="PSUM") as psA,
            tc.tile_pool(name="psB", bufs=2, space="PSUM") as psB,
        ):
            wd_t = cpool.tile([P, (R + 1) * P], bf)
            iota_t = cpool.tile([P, P], bf)
            eidx_t = cpool.tile([P, shared["eidx_w"]], i16)
            midx_t = cpool.tile([P, shared["midx_w"]], i16)
            nc.sync.dma_start(out=wd_t[:], in_=wd_d[:])
            nc.sync.dma_start(out=iota_t[:], in_=iota_d[:])
            nc.sync.dma_start(out=eidx_t[:], in_=eidx_d[:])
            nc.sync.dma_start(out=midx_t[:], in_=midx_d[:])
            iota_b = iota_t[:].rearrange("p (c f) -> p c f", c=1)

            # per-window emission, pass-A one window ahead of pass-B
            slabs_w = {}     # wv -> list of slab tiles

            def emit_passA(wv):
                ew = int(EWpad[wv])
                nops = _ceil(ew, GOP)
                ecol0 = int(EWpad[:wv].sum()) // 16
                slabs = []
                for k in range(nops):
                    nk = min(GOP, ew - k * GOP)
                    xts = xtp.tile([P, GOP], bf, tag="xts",
                                   name=f"xts_{wv}_{k}")
                    nc.gpsimd.dma_gather(
                        out_ap=xts[:, :nk].rearrange("p (c e) -> p c e", e=nk),
                        in_ap=xg_d[int(XB[wv]):, :],
                        idxs_ap=eidx_t[:, ecol0 + k * GOP // 16:
                                       ecol0 + k * GOP // 16 + nk // 16],
                        num_idxs=nk, num_idxs_reg=nk, elem_size=D,
                        single_packet=False, transpose=True,
                    )
                    slabs.append(xts)
                slabs_w[wv] = slabs
                # matmuls + staging copies + batched msgs writes
                chunks = p1[wv]
                ch0 = WIN_CH0[wv]
                i = 0
                while i < len(chunks):
                    nb = min(MB_CH, len(chunks) - i)
                    st = stg.tile([P, MB_CH * P], bf, tag="stage",
                                  name=f"stg_{wv}_{i}")
                    if SIM:
                        # sim shadow-memory: partial chunks (n<128) leave
                        # staging rows unwritten; harmless garbage on HW.
                        nc.vector.memset(st[:], 0.0)
                    for j in range(nb):
                        sl, scol, n, r = chunks[i + j]
                        mp = psA.tile([P, P], f32, tag="mp",
                                      name=f"mp_{wv}_{i}_{j}")
                        nc.tensor.matmul(
                            out=mp[:n, :],
                            lhsT=slabs[sl][:, scol:scol + n],
                            rhs=wd_t[:, r * P:(r + 1) * P],
                            start=True, stop=True,
                        )
                        nc.scalar.copy(out=st[:n, j * P:(j + 1) * P],
                                       in_=mp[:n, :])
                    base = (ch0 + i) * P
                    nc.sync.dma_start(
                        out=msgs_d[base:base + nb * P, :].rearrange(
                            "(c p) f -> p c f", p=P),
                        in_=st[:, :nb * P].rearrange("p (c f) -> p c f", f=P),
                    )
                    i += nb

            def emit_passB(wv):
                wch = win_chunks[wv]
                mcol0 = int(sum(win_slots[:wv])) // 16
                chb = int(sum(win_chunks[:wv]))
                meta_t = metap.tile([P, max_meta], bf, tag="meta",
                                    name=f"meta_{wv}")
                nc.sync.dma_start(
                    out=meta_t[:, :wch * 2].rearrange("p (c k) -> p c k", k=2),
                    in_=tgtw_d[chb:chb + wch].rearrange("c p k -> p c k"),
                )
                meta_b = meta_t[:].rearrange("p (c k) -> p c k", k=2)
                mslabs = []
                ms_tot = win_slots[wv]
                for k in range(_ceil(ms_tot, GOP)):
                    nk = min(GOP, ms_tot - k * GOP)
                    msl = msp.tile([P, GOP], bf, tag="msl",
                                   name=f"msl_{wv}_{k}")
                    nc.gpsimd.dma_gather(
                        out_ap=msl[:, :nk].rearrange("p (c e) -> p c e", e=D),
                        in_ap=msgs_d[WIN_CH0[wv] * P:WIN_CH0[wv + 1] * P, :],
                        idxs_ap=midx_t[:, mcol0 + k * GOP // 16:
                                       mcol0 + k * GOP // 16 + nk // 16],
                        num_idxs=nk, num_idxs_reg=nk, elem_size=D,
                        single_packet=False,
                    )
                    mslabs.append(msl)

                # batched S builds + per-tb accumulation
                ci = 0
                S_t = None
                for tbi in range(WIN_TB_START[wv], WIN_TB_START[wv + 1]):
                    K = int(chunks_tb[tbi])
                    acc = psB.tile([P, P], f32, tag="acc",
                                   name=f"acc_{wv}_{tbi}")
                    for k in range(K):
                        if ci % SBATCH == 0:
                            nbs = min(SBATCH, wch - ci)
                            S_t = spool.tile([P, SBATCH * P], bf, tag="S",
                                             name=f"S_{wv}_{ci}")
                            Sb = S_t[:, :nbs * P].rearrange(
                                "p (c f) -> p c f", f=P)
                            nc.vector.tensor_tensor(
                                out=Sb,
                                in0=iota_b.to_broadcast([P, nbs, P]),
                                in1=meta_b[:, ci:ci + nbs, 0:1].to_broadcast(
                                    [P, nbs, P]),
                                op=mybir.AluOpType.is_equal,
                            )
                            nc.vector.tensor_tensor(
                                out=Sb,
                                in0=Sb,
                                in1=meta_b[:, ci:ci + nbs, 1:2].to_broadcast(
                                    [P, nbs, P]),
                                op=mybir.AluOpType.mult,
                            )
                        sslot = ci % SBATCH
                        gpos = ci * P
                        sl, scol = gpos // GOP, gpos % GOP
                        nc.tensor.matmul(
                            out=acc[:],
                            lhsT=S_t[:, sslot * P:(sslot + 1) * P],
                            rhs=mslabs[sl][:, scol:scol + P],
                            start=(k == 0), stop=(k == K - 1),
                        )
                        ci += 1
                    ot = outp.tile([P, P], f32, tag="ot",
                                   name=f"ot_{wv}_{tbi}")
                    nc.scalar.copy(out=ot[:], in_=acc[:])
                    nc.sync.dma_start(out=out_d[tbi * P:(tbi + 1) * P, :],
                                      in_=ot[:])

            for wv in range(N_WIN):
                emit_passA(wv)
                if wv >= 1:
                    emit_passB(wv - 1)
            emit_passB(N_WIN - 1)
    nc.finalize()
    return nc


def kernel(x, blocks, edge_weights, source, target, edge_type):
    from concourse import bass_utils

    shared, in_maps = _preprocess(x, blocks, edge_weights, source, target,
                                  edge_type)
    nc = _build_nc(shared)
    if SIM:
        from concourse.bass_interp import CoreSim
        sim = CoreSim(nc, trace=False)
        for k, v in in_maps[0].items():
            sim.tensor(k)[:] = v
        sim.simulate()
        out = np.asarray(sim.tensor("out"))[:NT]
        # single core only in sim: return zeros elsewhere
        full = np.zeros((N_NODES, D), np.float32)
        full[:NT] = out
        kernel.last_exec_ns = None
        return full
    res = bass_utils.run_bass_kernel_spmd(
        nc, in_maps, core_ids=list(range(NCORES)), trace=TRACE,
    )
    out = np.concatenate([res.results[c]["out"][:NT] for c in range(NCORES)],
                         axis=0)
    if TRACE:
        kernel.last_exec_ns = res.exec_time_ns
    return out.astype(np.float32)


kernel.last_exec_ns = None
